# revision 19
# baseline (speedup 1.0000x reference)
"""CapacitiveMHA Trainium2 kernel.

Sharding: 8 cores = 4 batch shards x 2 head-group (tensor-parallel) shards.
Each core handles (batch b, heads [8g, 8g+8)): router+topk+gather replicated
per batch shard; q/kv/out projections and SDPA tensor-parallel over heads;
each core scatters its partial out-projection over the full sequence and the
host sums the two head-group partials per batch.

Schedule (per core): router scores run on the DVE (f32 multiply + free-axis
reduce per 128-row q tile), landing directly in the (128, 32) column-major
layout the top-k needs — no PE matmuls, no DRAM bounce.  KV projection is
split into two passes over vT (e-blocks 0-1 then 2-3) so SDPA for the first
head pair starts while K for the second pair is still being computed.  The
attention matmul is flipped (queries on psum partitions, head dim streamed)
which also yields per-head denominator columns for cheap normalization.
"""

import sys

sys.path.insert(0, "/opt/trn_rl_repo")

import numpy as np
import ml_dtypes

import concourse.bass as bass
import concourse.bacc as bacc
import concourse.mybir as mybir
import concourse.tile as tile
from concourse.bass_utils import run_bass_kernel_spmd

B, S, D = 4, 4096, 1024
H = 16          # total heads
HG = 2          # head groups (TP degree)
HC = H // HG    # heads per core = 8
dh = D // H     # 64
EH = D // HG    # e-range per core = 512
CAP = 512       # capacity
ROPE_BASE = 10000.0

dt = mybir.dt
F32, BF16, I32 = dt.float32, dt.bfloat16, dt.int32
F16 = dt.float16
AF = mybir.ActivationFunctionType
OP = mybir.AluOpType


def _bf16(x):
    return np.asarray(x, dtype=ml_dtypes.bfloat16)


def _build_program(reps=1):
    nc = bacc.Bacc()

    q_nat = nc.dram_tensor("q_nat", [S, D], F32, kind="ExternalInput")
    rw_rep = nc.dram_tensor("rw_rep", [128, D], F32, kind="ExternalInput")
    vT = nc.dram_tensor("vT", [D, S], BF16, kind="ExternalInput")
    wk = nc.dram_tensor("wk", [D, EH], BF16, kind="ExternalInput")
    wv = nc.dram_tensor("wv", [D, EH], BF16, kind="ExternalInput")
    wq = nc.dram_tensor("wq", [D, EH], BF16, kind="ExternalInput")
    ow = nc.dram_tensor("ow", [EH, D], BF16, kind="ExternalInput")
    fkT = nc.dram_tensor("fkT", [S, dh], F32, kind="ExternalInput")
    fkrep = nc.dram_tensor("fkrep", [128, S], BF16, kind="ExternalInput")
    iotah_d = nc.dram_tensor("iotah", [128, 32], F16, kind="ExternalInput")
    iotal_d = nc.dram_tensor("iotal", [128, 32], F16, kind="ExternalInput")
    iota512_d = nc.dram_tensor("iota512", [128, 512], F16, kind="ExternalInput")
    ones1x128_d = nc.dram_tensor("ones1x128", [1, 128], F32, kind="ExternalInput")
    u32_d = nc.dram_tensor("u32", [32, 32], F32, kind="ExternalInput")
    l128_d = nc.dram_tensor("l128", [128, 128], F32, kind="ExternalInput")
    ident_d = nc.dram_tensor("ident", [128, 128], F32, kind="ExternalInput")
    rep64_d = nc.dram_tensor("rep64", [64, 128], F32, kind="ExternalInput")
    identb_d = nc.dram_tensor("identb", [128, 128], BF16, kind="ExternalInput")

    out_ext = nc.dram_tensor("out", [S, D], F32, kind="ExternalOutput")

    t = dict(locals())
    with tile.TileContext(nc) as tc:
        for _ in range(reps):
            _body(nc, tc, t)
    nc.compile()
    return nc


def _body(nc, tc, t):
    q_nat, vT = t["q_nat"], t["vT"]
    rw_rep = t["rw_rep"]
    wk, wv, wq, ow = t["wk"], t["wv"], t["wq"], t["ow"]
    fkT, fkrep = t["fkT"], t["fkrep"]
    out_ext = t["out_ext"]

    with (
        tc.tile_pool(name="const", bufs=1) as cp,
        tc.tile_pool(name="persist", bufs=1) as pp,
        tc.tile_pool(name="scratch", bufs=2) as scr,
        tc.tile_pool(name="respool", bufs=1) as resp,
    ):
        # ---- constants (Pool/SWDGE queue) ----
        ident = cp.tile([128, 128], F32, name="ident", tag="ident")
        nc.gpsimd.dma_start(ident[:], t["ident_d"][:])
        identb = cp.tile([128, 128], BF16, name="identb", tag="identb")
        nc.gpsimd.dma_start(identb[:], t["identb_d"][:])
        rw_rep_sb = cp.tile([128, D], F32, name="rw_rep_sb", tag="rw_rep_sb")
        nc.gpsimd.dma_start(rw_rep_sb[:], rw_rep[:])
        iotah = cp.tile([128, 32], F16, name="iotah", tag="iotah")
        nc.gpsimd.dma_start(iotah[:], t["iotah_d"][:])
        iotal = cp.tile([128, 32], F16, name="iotal", tag="iotal")
        nc.gpsimd.dma_start(iotal[:], t["iotal_d"][:])
        iota512 = cp.tile([128, 512], F16, name="iota512", tag="iota512")
        nc.gpsimd.dma_start(iota512[:], t["iota512_d"][:])
        ones1x128 = cp.tile([1, 128], F32, name="ones1x128", tag="ones1x128")
        nc.gpsimd.dma_start(ones1x128[:], t["ones1x128_d"][:])
        u32c = cp.tile([32, 32], F32, name="u32c", tag="u32c")
        nc.gpsimd.dma_start(u32c[:], t["u32_d"][:])
        l128c = cp.tile([128, 128], F32, name="l128c", tag="l128c")
        nc.gpsimd.dma_start(l128c[:], t["l128_d"][:])
        rep64c = cp.tile([64, 128], F32, name="rep64c", tag="rep64c")
        nc.gpsimd.dma_start(rep64c[:], t["rep64_d"][:])
        fkrep_sb = pp.tile([128, S], BF16, name="fkrep_sb", tag="fkrep_sb")
        nc.gpsimd.dma_start(fkrep_sb[:], fkrep[:])

        # ---- weights: wk/wv on the SP ring (feed kv first) ----
        wk_sb, wv_sb = [], []
        for d in range(8):
            tk = pp.tile([128, EH], BF16, name=f"wk{d}", tag=f"wk{d}")
            nc.sync.dma_start(tk[:], wk[128 * d:128 * (d + 1), :])
            wk_sb.append(tk)
            tv = pp.tile([128, EH], BF16, name=f"wv{d}", tag=f"wv{d}")
            nc.sync.dma_start(tv[:], wv[128 * d:128 * (d + 1), :])
            wv_sb.append(tv)
        # wq/ow on the SP ring behind the query stream (ACT queue carries the
        # vT stream, which drains before the exp phase needs the engine)
        wq_sb, ow_sb = [], []
        for d in range(8):
            tq = pp.tile([128, EH], BF16, name=f"wq{d}", tag=f"wq{d}")
            nc.sync.dma_start(tq[:], wq[128 * d:128 * (d + 1), :])
            wq_sb.append(tq)
        for e in range(4):
            to = pp.tile([128, D], BF16, name=f"ow{e}", tag=f"ow{e}")
            nc.sync.dma_start(to[:], ow[128 * e:128 * (e + 1), :])
            ow_sb.append(to)

        # persistent activations
        kT_sb = [pp.tile([128, S], BF16, name=f"kT{e}", tag=f"kT{e}") for e in range(4)]
        qT_sb = [pp.tile([128, 512], BF16, name=f"qTt{e}", tag=f"qTt{e}") for e in range(4)]
        attT_sb = [pp.tile([128, 512], BF16, name=f"att{e}", tag=f"att{e}") for e in range(4)]
        v_sb = pp.tile([128, 520 * 32], BF16, name="v_sb", tag="v_sb")  # 32 chunks x (8h x 65)

        # streaming pools for vT (kv) and qT (router) + kv psum; opened up
        # front so the scheduler can overlap kv-proj with the router/topk
        # phases (psum banks must not be reused across the overlap window)
        vstream_cm = tc.tile_pool(name="vstream", bufs=16)
        vstreamp = vstream_cm.__enter__()
        pk_pool_cm = tc.tile_pool(name="pk", bufs=1, space="PSUM")
        pkp = pk_pool_cm.__enter__()
        pv_pool_cm = tc.tile_pool(name="pv", bufs=1, space="PSUM")
        pvp = pv_pool_cm.__enter__()
        qblk_cm = tc.tile_pool(name="qblk", bufs=2)
        qblkp = qblk_cm.__enter__()

        # ===== Phase A: router scores (DVE fused mult+reduce, f32) =====
        # q_nat tile t (rows 128t..128t+127) reduces to column t of the
        # (128, 32) column-major scores layout the top-k needs — no PE, no
        # DRAM bounce.
        scores_cm = pp.tile([128, 32], F32, name="scores_cm", tag="scores_cm")
        for tq in range(32):
            qb = qblkp.tile([128, D], F32, name="qb", tag="qb")
            nc.sync.dma_start(qb[:], q_nat[128 * tq:128 * (tq + 1), :])
            ttr = qblkp.tile([128, D], F32, name="ttr", tag="ttr")
            nc.gpsimd.tensor_tensor(ttr[:], qb[:], rw_rep_sb[:], op=OP.mult)
            nc.vector.tensor_reduce(
                scores_cm[:, tq:tq + 1], ttr[:], mybir.AxisListType.X, op=OP.add
            )

        # =========== Phase B: top-512 threshold + compaction ===========
        idx_col = []
        topw_col = []
        with tc.tile_pool(name="pb", bufs=1, space="PSUM") as pbp:
            # selection threshold = the 512th-largest score, found by
            # branchless f32 bisection on cnt(scores >= t) (the gpsimd
            # kth_largest ucode costs ~680us on HW — 20x this loop).
            # Invariant: cnt(>= lo) >= 512 > cnt(>= hi); all state is
            # replicated [128,1] so the is_ge mask needs no broadcast.
            # Interval width halves deterministically, so only lo is state:
            # mid = lo + 2^-k, lo += ge * 2^-k with immediate half-widths.
            # 24 rounds: final width 32/2^24 = 1.9e-6, 180x below the
            # smallest rank-512 gap (3.5e-4) of randn-seeded scores; every
            # lo is an exact f32 sum of powers of two.
            ones128 = scr.tile([128, 128], F32, name="ones128", tag="ones128")
            nc.vector.memset(ones128[:], 1.0)
            lo_t = scr.tile([128, 1], F32, name="lo_t", tag="lo_t")
            nc.vector.memset(lo_t[:], -16.0)
            mid_t = scr.tile([128, 1], F32, name="mid_t", tag="mid_t")
            bmask = scr.tile([128, 32], F32, name="bmask", tag="bmask")
            rowc = scr.tile([128, 1], F32, name="rowc", tag="rowc")
            ge_t = scr.tile([128, 1], F32, name="ge_t", tag="ge_t")
            for it in range(24):
                halfw = 16.0 / (1 << it)
                nc.vector.tensor_scalar(mid_t[:], lo_t[:], halfw, None,
                                        op0=OP.add)
                nc.vector.tensor_scalar(bmask[:], scores_cm[:], mid_t[:], None,
                                        op0=OP.is_ge)
                nc.vector.tensor_reduce(rowc[:], bmask[:],
                                        mybir.AxisListType.X, op=OP.add)
                pc = pbp.tile([128, 1], F32)
                nc.tensor.matmul(pc[:], lhsT=ones128[:], rhs=rowc[:],
                                 start=True, stop=True)
                nc.vector.tensor_scalar(ge_t[:], pc[:], float(CAP), None,
                                        op0=OP.is_ge)
                nc.vector.tensor_scalar(ge_t[:], ge_t[:], halfw, None,
                                        op0=OP.mult)
                nc.vector.tensor_tensor(lo_t[:], lo_t[:], ge_t[:], op=OP.add)
            thr_col = lo_t

            # masks (mask in (128,32) and transposed (32,128))
            mask = pp.tile([128, 32], F32, name="mask", tag="mask")
            nc.vector.tensor_scalar(mask[:], scores_cm[:], thr_col[:], None,
                                    op0=OP.is_ge)
            pst = pbp.tile([32, 128], F32)
            nc.tensor.transpose(pst[:], scores_cm[:], ident[:])
            s_T = scr.tile([32, 128], F32, name="s_T", tag="s_T")
            nc.vector.tensor_copy(s_T[:], pst[:])
            mask_T = scr.tile([32, 128], F32, name="mask_T", tag="mask_T")
            nc.vector.tensor_scalar(mask_T[:], s_T[:], thr_col[:32, :], None,
                                    op0=OP.is_ge)

            # in-row inclusive prefix: pfx = mask_T.T @ U32  -> (128, 32)
            ppfx = pbp.tile([128, 32], F32)
            nc.tensor.matmul(ppfx[:], lhsT=mask_T[:], rhs=u32c[:],
                             start=True, stop=True)
            pfx = scr.tile([128, 32], F32, name="pfx", tag="pfx")
            nc.vector.tensor_copy(pfx[:], ppfx[:])
            # cross-row exclusive prefix of row sums: S = L128.T @ rowsum
            pS = pbp.tile([128, 1], F32)
            nc.tensor.matmul(pS[:], lhsT=l128c[:], rhs=pfx[:, 31:32],
                             start=True, stop=True)
            Scol = scr.tile([128, 1], F32, name="Scol", tag="Scol")
            nc.vector.tensor_copy(Scol[:], pS[:])

            rank = scr.tile([128, 32], F32, name="rank", tag="rank")
            nc.vector.tensor_tensor(rank[:], pfx[:], Scol[:].to_broadcast([128, 32]),
                                    op=OP.add)
            nc.vector.tensor_tensor(rank[:], rank[:], mask[:], op=OP.subtract)
            # rank_eff = mask ? rank : 512  (= (rank-512)*mask + 512)
            nc.vector.tensor_scalar(rank[:], rank[:], float(CAP), None,
                                    op0=OP.subtract)
            nc.vector.tensor_tensor(rank[:], rank[:], mask[:], op=OP.mult)
            nc.vector.tensor_scalar(rank[:], rank[:], float(CAP), None, op0=OP.add)

            # fp16 split of scores: s = shi + slo (each fp16-exact)
            shi = scr.tile([128, 32], F16, name="shi", tag="shi")
            nc.vector.tensor_copy(shi[:], scores_cm[:])
            slo = scr.tile([128, 32], F16, name="slo", tag="slo")
            nc.vector.tensor_tensor(slo[:], scores_cm[:], shi[:], op=OP.subtract)
            # fp16 rank copy for 2x-mode one-hot compares
            rank16 = scr.tile([128, 32], F16, name="rank16", tag="rank16")
            nc.vector.tensor_copy(rank16[:], rank[:])

            # combo tile: interleave [ihi | ilo | shi | slo] quads -> (128, 128)
            combo = scr.tile([128, 128], F16, name="combo", tag="combo")
            for ci, srct in enumerate((iotah, iotal, shi, slo)):
                nc.vector.tensor_copy(
                    combo[:].rearrange("p (c four) -> p c four", four=4)
                    [:, :, ci:ci + 1],
                    srct[:].rearrange("p (c one) -> p c one", one=1),
                )

            # one-hot P tiles + [idx_hi; idx_lo; w_hi; w_lo] extraction
            piw = pbp.tile([4, 512], F32)
            for c in range(32):
                Pc = scr.tile([128, 512], F16, name="Pc", tag="Pc")
                nc.vector.tensor_tensor(
                    Pc[:], rank16[:, c:c + 1].to_broadcast([128, 512]), iota512[:],
                    op=OP.is_equal,
                )
                nc.tensor.matmul(piw[:], lhsT=combo[:, 4 * c:4 * c + 4], rhs=Pc[:],
                                 start=(c == 0), stop=(c == 31))
            iw_sb = scr.tile([4, 512], F32, name="iw_sb", tag="iw_sb")
            nc.vector.tensor_copy(iw_sb[:], piw[:])

            # transpose to column layout (4 chunks of 128)
            for j in range(4):
                pt = pbp.tile([128, 4], F32)
                nc.tensor.transpose(pt[:], iw_sb[:, 128 * j:128 * (j + 1)],
                                    ident[:4, :4])
                iwT = pp.tile([128, 4], F32, name=f"iwT{j}", tag=f"iwT{j}")
                nc.vector.tensor_copy(iwT[:], pt[:])
                # idx = 64*hi + lo ; w = whi + wlo
                nc.vector.tensor_scalar(iwT[:, 0:1], iwT[:, 0:1], 64.0, None,
                                        op0=OP.mult)
                nc.vector.tensor_tensor(iwT[:, 0:1], iwT[:, 0:1], iwT[:, 1:2],
                                        op=OP.add)
                nc.vector.tensor_tensor(iwT[:, 2:3], iwT[:, 2:3], iwT[:, 3:4],
                                        op=OP.add)
                ic = pp.tile([128, 1], I32, name=f"idxc{j}", tag=f"idxc{j}")
                nc.vector.tensor_copy(ic[:], iwT[:, 0:1])
                idx_col.append(ic)
                topw_col.append(iwT)

        # =========== Phase C: gather + q-proj + rope-q ===========
        with tc.tile_pool(name="pc", bufs=1, space="PSUM") as pcp:
            # critical path first: gather query rows (cast to bf16 in SWDGE)
            res = [resp.tile([128, D], BF16, name=f"res{j}", tag=f"res{j}") for j in range(4)]
            for j in range(4):
                nc.gpsimd.indirect_dma_start(
                    out=res[j][:], out_offset=None, in_=q_nat[:],
                    in_offset=bass.IndirectOffsetOnAxis(ap=idx_col[j][:, 0:1], axis=0),
                )
            # transpose resampled -> rT (d-part, c-free), bf16
            rT_sb = []
            for d in range(8):
                prt = pcp.tile([128, 512], BF16, name="prt", tag="prt")
                for j in range(4):
                    nc.tensor.transpose(
                        prt[:, 128 * j:128 * (j + 1)],
                        res[j][:, 128 * d:128 * (d + 1)], identb[:],
                    )
                rt = pp.tile([128, 512], BF16, name=f"rT{d}", tag=f"rT{d}")
                nc.vector.tensor_copy(rt[:], prt[:])
                rT_sb.append(rt)

            # rope-q factor: gather fkT rows then transpose into (128, 512)
            pfq = pcp.tile([64, 512], F32, name="pfq", tag="pfq")
            for j in range(4):
                fqg = scr.tile([128, 64], F32, name="fqg", tag="fqg")
                nc.gpsimd.indirect_dma_start(
                    out=fqg[:], out_offset=None, in_=fkT[:],
                    in_offset=bass.IndirectOffsetOnAxis(ap=idx_col[j][:, 0:1], axis=0),
                )
                nc.tensor.transpose(pfq[:, 128 * j:128 * (j + 1)], fqg[:],
                                    ident[:])
            fq_half = scr.tile([64, 512], F32, name="fq_half", tag="fq_half")
            nc.vector.tensor_copy(fq_half[:], pfq[:])
            pfq2 = pcp.tile([128, 512], F32, name="pfq2", tag="pfq2")
            nc.tensor.matmul(pfq2[:], lhsT=rep64c[:], rhs=fq_half[:],
                             start=True, stop=True)
            fq_rep = pp.tile([128, 512], F32, name="fq_rep", tag="fq_rep")
            nc.vector.tensor_copy(fq_rep[:], pfq2[:])

            # q-proj (+rope) -> qT_sb
            for e in range(4):
                pq = pcp.tile([128, 512], F32, name="pq", tag="pq")
                for d in range(8):
                    nc.tensor.matmul(
                        pq[:], lhsT=wq_sb[d][:, 128 * e:128 * (e + 1)],
                        rhs=rT_sb[d][:], start=(d == 0), stop=(d == 7),
                    )
                nc.vector.tensor_tensor(qT_sb[e][:], pq[:], fq_rep[:], op=OP.mult)

        # =========== Phase D: kv-proj, two passes over vT ===========
        # pass 0: K for e-blocks {0,1} + all of V;  pass 1: K for {2,3}.
        # V is produced s-major into v_sb (all 8 heads per s-chunk).
        def _kv_pass(p, vstream):
            for q in range(4):            # vT quarters of 1024 positions
                vblk = []
                for d in range(8):
                    blk = vstream.tile([128, 1024], BF16, name="vstr",
                                       tag="vstr")
                    nc.scalar.dma_start(
                        blk[:],
                        vT[128 * d:128 * (d + 1), 1024 * q:1024 * (q + 1)],
                    )
                    vblk.append(blk)
                for sch in range(2):
                    sc = 2 * q + sch
                    for e in (2 * p, 2 * p + 1):
                        pk = pkp.tile([128, 512], F32)
                        for d in range(8):
                            nc.tensor.matmul(
                                pk[:], lhsT=wk_sb[d][:, 128 * e:128 * (e + 1)],
                                rhs=vblk[d][:, 512 * sch:512 * (sch + 1)],
                                start=(d == 0), stop=(d == 7),
                            )
                        nc.vector.tensor_tensor(
                            kT_sb[e][:, 512 * sc:512 * (sc + 1)], pk[:],
                            fkrep_sb[:, 512 * sc:512 * (sc + 1)], op=OP.mult,
                        )
                    if p == 0:
                        for q4 in range(4):
                            pv = pvp.tile([128, 512], F32)
                            for d in range(8):
                                nc.tensor.matmul(
                                    pv[:],
                                    lhsT=vblk[d][:, 512 * sch + 128 * q4:
                                                 512 * sch + 128 * (q4 + 1)],
                                    rhs=wv_sb[d][:], start=(d == 0), stop=(d == 7),
                                )
                            base = 520 * (4 * sc + q4)
                            nc.vector.tensor_copy(
                                v_sb[:, base:base + 520]
                                .rearrange("p (h c) -> p h c", h=8)[:, :, 0:64],
                                pv[:].rearrange("p (h c) -> p h c", h=8),
                            )
                            nc.vector.memset(
                                v_sb[:, base:base + 520]
                                .rearrange("p (h c) -> p h c", h=8)[:, :, 64:65],
                                1.0,
                            )

        qblk_cm.__exit__(None, None, None)
        _kv_pass(0, vstreamp)
        pv_pool_cm.__exit__(None, None, None)
        _kv_pass(1, vstreamp)
        pk_pool_cm.__exit__(None, None, None)
        vstream_cm.__exit__(None, None, None)

        # =========== Phase E: SDPA, flipped att (per e-block) ===========
        # patt[hh][cc, 65j+dd] = sum_s exp(score) * v  (col 64 = denom)
        with (
            tc.tile_pool(name="psc", bufs=3, space="PSUM") as pscp,
            tc.tile_pool(name="patt", bufs=1, space="PSUM") as pattp,
            tc.tile_pool(name="epool", bufs=3) as ep,
            tc.tile_pool(name="apool", bufs=2) as ap_,
        ):
            for e in range(4):
                patt = [pattp.tile([65, 512], F32, name=f"patt{hh}",
                                   tag=f"patt{hh}") for hh in range(2)]
                for tch in range(32):
                    psc = pscp.tile([128, 1024], F32)
                    for hh in range(2):
                        nc.tensor.matmul(
                            psc[:, 512 * hh:512 * (hh + 1)],
                            lhsT=kT_sb[e][64 * hh:64 * (hh + 1),
                                          128 * tch:128 * (tch + 1)],
                            rhs=qT_sb[e][64 * hh:64 * (hh + 1), :],
                            start=True, stop=True,
                        )
                    et = ep.tile([128, 1024], BF16, name="et", tag="et")
                    nc.scalar.activation(et[:], psc[:], AF.Exp)
                    for hh in range(2):
                        vb = 520 * tch + 65 * (2 * e + hh)
                        nc.tensor.matmul(
                            patt[hh][:],
                            lhsT=v_sb[:, vb:vb + 65],
                            rhs=et[:, 512 * hh:512 * (hh + 1)],
                            start=(tch == 0), stop=(tch == 31),
                        )
                for hh in range(2):
                    recip = scr.tile([1, 512], F32, name="recip", tag="recip",
                                     bufs=1)
                    nc.vector.reciprocal(recip[:], patt[hh][64:65, :])
                    rrep = scr.tile([64, 512], F32, name="rrep", tag="rrep",
                                    bufs=1)
                    nc.gpsimd.partition_broadcast(rrep[:], recip[:], channels=64)
                    nc.vector.tensor_tensor(
                        attT_sb[e][64 * hh:64 * (hh + 1), :],
                        patt[hh][0:64, :], rrep[:], op=OP.mult,
                    )

        # =========== Phase F: out-proj + scale + scatter ===========
        with (
            tc.tile_pool(name="po", bufs=2, space="PSUM") as pop,
            tc.tile_pool(name="opool", bufs=2) as op_,
        ):
            for j in range(4):
                po = pop.tile([128, 1024], F32)
                for e in range(4):
                    for k in range(2):
                        nc.tensor.matmul(
                            po[:, 512 * k:512 * (k + 1)],
                            lhsT=attT_sb[e][:, 128 * j:128 * (j + 1)],
                            rhs=ow_sb[e][:, 512 * k:512 * (k + 1)],
                            start=(e == 0), stop=(e == 3),
                        )
                osb = op_.tile([128, 1024], F32, name="osb", tag="osb")
                nc.vector.tensor_scalar(osb[:], po[:], topw_col[j][:, 2:3],
                                        None, op0=OP.mult)
                nc.gpsimd.indirect_dma_start(
                    out=out_ext[:],
                    out_offset=bass.IndirectOffsetOnAxis(ap=idx_col[j][:, 0:1], axis=0),
                    in_=osb[:], in_offset=None,
                )


_NC_CACHE = None


def _get_nc():
    global _NC_CACHE
    if _NC_CACHE is None:
        _NC_CACHE = _build_program()
    return _NC_CACHE


def _host_constants():
    pos = np.arange(S, dtype=np.float32)
    freqs = np.exp(
        np.linspace(0.0, -1.0, dh // 2, dtype=np.float32)
        * np.log(np.float32(ROPE_BASE))
    ).astype(np.float32)
    angles = pos[:, None] * freqs[None, :]          # (S, 32) f32
    fkT = np.concatenate([np.sin(angles), np.cos(angles)], axis=1).astype(
        np.float32
    )                                               # (S, 64)
    fk_scaled = (fkT.T / np.float32(8.0)).astype(np.float32)   # (64, S)
    fkrep = np.concatenate([fk_scaled, fk_scaled], axis=0)     # (128, S)

    p = np.arange(128)[:, None]
    c = np.arange(32)[None, :]
    iota_cm = (128 * c + p).astype(np.float32)
    iotah = (iota_cm // 64).astype(np.float16)
    iotal = (iota_cm % 64).astype(np.float16)
    iota512 = np.tile(np.arange(512, dtype=np.float16)[None, :], (128, 1))
    return dict(
        fkT=fkT, fkrep=_bf16(np.ascontiguousarray(fkrep)),
        iotah=iotah, iotal=iotal, iota512=iota512,
        ones1x128=np.ones((1, 128), np.float32),
        u32=np.triu(np.ones((32, 32), np.float32)),
        rep64=np.tile(np.eye(64, dtype=np.float32), (1, 2)),
        identb=_bf16(np.eye(128, dtype=np.float32)),
        l128=np.triu(np.ones((128, 128), np.float32), k=1),
        ident=np.eye(128, dtype=np.float32),
    )


def make_in_maps(query_seq, value_seq, router_w, q_w, kv_w, out_w):
    query_seq = np.asarray(query_seq, np.float32)
    value_seq = np.asarray(value_seq, np.float32)
    router_w = np.asarray(router_w, np.float32)
    q_w = np.asarray(q_w, np.float32)
    kv_w = np.asarray(kv_w, np.float32)
    out_w = np.asarray(out_w, np.float32)

    consts = _host_constants()
    rw_rep = np.ascontiguousarray(np.tile(router_w.reshape(1, D), (128, 1)))

    in_maps = []
    for core in range(8):
        b, g = core // 2, core % 2
        es = slice(EH * g, EH * (g + 1))
        m = dict(
            q_nat=np.ascontiguousarray(query_seq[b]),
            rw_rep=rw_rep,
            vT=_bf16(np.ascontiguousarray(value_seq[b].T)),
            wk=_bf16(np.ascontiguousarray(kv_w[es, :].T)),
            wv=_bf16(np.ascontiguousarray(kv_w[D + EH * g:D + EH * (g + 1), :].T)),
            wq=_bf16(np.ascontiguousarray(q_w[es, :].T)),
            ow=_bf16(np.ascontiguousarray(out_w[:, es].T)),
            **consts,
        )
        in_maps.append(m)
    return in_maps


def kernel(query_seq, value_seq, router_w, q_w, kv_w, out_w):
    nc = _get_nc()
    in_maps = make_in_maps(query_seq, value_seq, router_w, q_w, kv_w, out_w)
    try:
        res = run_bass_kernel_spmd(nc, in_maps, list(range(8))).results
    except Exception:
        # transient NRT_EXEC_UNIT_UNRECOVERABLE from a prior wedged session
        # clears on the next dispatch; retry once
        res = run_bass_kernel_spmd(nc, in_maps, list(range(8))).results
    out = np.stack(
        [
            res[2 * b]["out"].astype(np.float32)
            + res[2 * b + 1]["out"].astype(np.float32)
            for b in range(B)
        ]
    )
    return out

